# revision 3
# baseline (speedup 1.0000x reference)
"""Trainium2 Bass kernel for NeighborStatOP (retrieval_knn).

Computes, for each frame and each local atom i:
  min_rr2[f, i]  = min_{j != i} |x_j - x_i|^2                      (f32)
  max_nnei[f, t] = max_i #{ j != i : |x_j - x_i|^2 < 6^2, type_j = t } (int32)

Strategy (8 NeuronCores, SPMD, one compiled program):
  - Shard query atoms: core c handles frame c//4, queries (c%4)*1024..+1024.
  - Atoms are permuted host-side so keys are grouped by type (stable sort);
    the same permutation applies to queries, so the self-pair stays on the
    "diagonal" block of the (query, key) distance matrix.
  - rr2 is computed on the tensor engine via the centered per-component
    expansion  qc^2 - 2*qc*kc + kc^2, with every fp32 feature split into
    3 bf16 pieces (products of bf16 pairs are exact in fp32, PSUM
    accumulation is fp32).  K = 36 rows, full-rate bf16 matmuls.
  - The self-pair is knocked out by accumulating BIG (1e30) onto the
    diagonal through a tiny extra matmul: lhsT = identity(128),
    rhs = per-core diagmask block (BIG diag at the core's own candidate
    position, zeros at the other three), start=False.  The program is
    identical on all cores; the data carries the position.
  - min: DVE tensor_reduce(min) over each PSUM half (128 x 2048).
  - per-type counts: ScalarE Sign activation passes over type-segment pieces
    with accum_out; count_t = (sum(sign(36-rr2)) + width) / 2 on host
    (the BIG self entry yields sign -1, excluding itself automatically).
    Pieces are cut at the union of both frames' type boundaries so a single
    program works for all cores.
  - Tiny finalization (piece sums, inverse permutation, max) on host.
"""
import sys

sys.path.insert(0, "/opt/trn_rl_repo")

import numpy as np
import ml_dtypes

NFRAMES = 2
NLOC = 4096
NTYPES = 4
RCUT2 = 36.0
CENTER = 20.0
BIG = 1.0e30
NCORES = 8
CPF = 4                       # cores per frame
QPC = NLOC // CPF             # queries per core = 1024
NQT = QPC // 128              # query tiles per core = 8
HALF = 2048                   # PSUM half width (4 banks)
PAIRS = [(0, 0), (0, 1), (1, 0), (1, 1), (0, 2), (2, 0)]
K = 36                        # 12 split rows per component

_CACHE = {}


def _split3(x):
    p1 = x.astype(ml_dtypes.bfloat16)
    r1 = (x.astype(np.float64) - p1.astype(np.float64)).astype(np.float32)
    p2 = r1.astype(ml_dtypes.bfloat16)
    r2 = (r1.astype(np.float64) - p2.astype(np.float64)).astype(np.float32)
    p3 = r2.astype(ml_dtypes.bfloat16)
    return [p1, p2, p3]


def _features(coords):
    """coords: (n, 3) float64 centered.
    Returns (qfeat (K, n), kfeat (K, n)) as bfloat16 arrays."""
    n = len(coords)
    ones = np.ones(n, ml_dtypes.bfloat16)
    zer = np.zeros(n, ml_dtypes.bfloat16)
    L, R = [], []
    for d in range(3):
        qc = coords[:, d]
        q2 = (qc * qc).astype(np.float32)
        q2p = _split3(q2)
        qp = _split3(qc.astype(np.float32))
        m2p = _split3((-2.0 * qc).astype(np.float32))
        for i in range(3):              # qc^2 pieces x 1
            L.append(q2p[i]); R.append(ones)
        for (ia, ib) in PAIRS:          # qc pieces x -2kc pieces
            L.append(qp[ia]); R.append(m2p[ib])
        for i in range(3):              # 1 x kc^2 pieces
            L.append(ones); R.append(q2p[i])
    del zer
    return (np.stack(L).astype(ml_dtypes.bfloat16),
            np.stack(R).astype(ml_dtypes.bfloat16))


def _pieces(bounds):
    """Split [0, NLOC) at sorted interior bounds + the HALF cut.
    Returns list per half: list of (a, b) absolute column ranges."""
    cuts = sorted(set(list(bounds) + [HALF]) - {0, NLOC})
    edges = [0] + cuts + [NLOC]
    per_half = [[], []]
    for a, b in zip(edges[:-1], edges[1:]):
        per_half[0 if a < HALF else 1].append((a, b))
    return per_half


def _build(pieces_per_half, repeat=1):
    """Build + lower the SPMD kernel."""
    import concourse.bacc as bacc
    import concourse.tile as tile
    from concourse import mybir

    f32 = mybir.dt.float32
    bf16 = mybir.dt.bfloat16
    NP = sum(len(p) for p in pieces_per_half)

    nc = bacc.Bacc("TRN2", target_bir_lowering=False, debug=False,
                   num_devices=NCORES)
    qf = nc.dram_tensor("qfeat", [K, QPC], bf16, kind="ExternalInput").ap()
    kf = nc.dram_tensor("kfeat", [K, NLOC], bf16, kind="ExternalInput").ap()
    dm = nc.dram_tensor("diagmask", [128, CPF * 128], bf16,
                        kind="ExternalInput").ap()
    idn = nc.dram_tensor("ident", [128, 128], bf16, kind="ExternalInput").ap()
    out_min = nc.dram_tensor("out_min", [128, NQT * 2], f32,
                             kind="ExternalOutput").ap()
    out_cnt = nc.dram_tensor("out_cnt", [128, NQT * NP], f32,
                             kind="ExternalOutput").ap()

    with tile.TileContext(nc) as tc:
        with (
            tc.tile_pool(name="singles", bufs=1) as singles,
            tc.tile_pool(name="psum", bufs=2, space="PSUM") as psum_pool,
            tc.tile_pool(name="scratch", bufs=2) as scratch_pool,
        ):
            qsb = singles.tile([K, QPC], bf16)
            nc.sync.dma_start(out=qsb[:], in_=qf)
            ksb = singles.tile([K, NLOC], bf16)
            nc.sync.dma_start(out=ksb[:], in_=kf)
            dmsb = singles.tile([128, CPF * 128], bf16)
            nc.sync.dma_start(out=dmsb[:], in_=dm)
            idsb = singles.tile([128, 128], bf16)
            nc.sync.dma_start(out=idsb[:], in_=idn)
            bias36 = singles.tile([128, 1], f32)
            nc.vector.memset(bias36[:], RCUT2)
            min_sb = singles.tile([128, NQT * 2], f32)
            cnt_sb = singles.tile([128, NQT * NP], f32)

            def body(_iv=None):
                for t in range(NQT):
                    pi_global = 0
                    for h in range(2):
                        ps = psum_pool.tile([128, HALF], f32, tag="ps")
                        # diag candidate cols within this half:
                        #   pos 2h   -> t*128;  pos 2h+1 -> 1024 + t*128
                        diag_of_block = {
                            (t * 128) // 512: (2 * h, t * 128),
                            (1024 + t * 128) // 512: (2 * h + 1,
                                                      1024 + t * 128),
                        }
                        for b in range(HALF // 512):
                            c0 = h * HALF + b * 512
                            hasd = b in diag_of_block
                            nc.tensor.matmul(
                                ps[:, b * 512:(b + 1) * 512],
                                lhsT=qsb[:, t * 128:(t + 1) * 128],
                                rhs=ksb[:, c0:c0 + 512],
                                start=True, stop=not hasd,
                            )
                            if hasd:
                                pos, rel = diag_of_block[b]
                                nc.tensor.matmul(
                                    ps[:, rel:rel + 128],
                                    lhsT=idsb[:],
                                    rhs=dmsb[:, pos * 128:(pos + 1) * 128],
                                    start=False, stop=True,
                                    skip_group_check=True,
                                )
                        mcol = t * 2 + h
                        nc.vector.tensor_reduce(
                            out=min_sb[:, mcol:mcol + 1], in_=ps[:],
                            axis=mybir.AxisListType.X, op=mybir.AluOpType.min,
                        )
                        sc = scratch_pool.tile([128, HALF], f32, tag="sc")
                        for (a, b_) in pieces_per_half[h]:
                            ccol = t * NP + pi_global
                            nc.scalar.activation(
                                sc[:, a - h * HALF:b_ - h * HALF],
                                ps[:, a - h * HALF:b_ - h * HALF],
                                mybir.ActivationFunctionType.Sign,
                                bias=bias36[:], scale=-1.0,
                                accum_out=cnt_sb[:, ccol:ccol + 1],
                            )
                            pi_global += 1

            if repeat == 1:
                body()
            else:
                with tc.For_i(0, repeat, 1) as iv:
                    body(iv)

            nc.sync.dma_start(out=out_min, in_=min_sb[:])
            nc.sync.dma_start(out=out_cnt, in_=cnt_sb[:])

    nc.compile()
    return nc, NP


def _prep(coord, atype):
    """Host-side prep. Returns (in_maps, perms, pieces_per_half)."""
    c = np.asarray(coord, dtype=np.float32).reshape(NFRAMES, NLOC, 3)
    at = np.asarray(atype)
    perms, kfeats, qfeats, bounds_all = [], [], [], set()
    for f in range(NFRAMES):
        perm = np.argsort(at[f], kind="stable")
        perms.append(perm)
        cs = c[f][perm].astype(np.float64) - CENTER
        qfeat, kfeat = _features(cs)
        qfeats.append(qfeat)
        kfeats.append(kfeat)
        counts = np.bincount(at[f], minlength=NTYPES)
        bounds_all.update(np.cumsum(counts)[:-1].tolist())
    pieces_per_half = _pieces(bounds_all)

    bigdiag = np.zeros((128, 128), np.float32)
    np.fill_diagonal(bigdiag, BIG)
    bigdiag = bigdiag.astype(ml_dtypes.bfloat16)
    ident = np.eye(128, dtype=ml_dtypes.bfloat16)

    in_maps = []
    for core in range(NCORES):
        f, s = core // CPF, core % CPF
        dmask = np.zeros((128, CPF * 128), ml_dtypes.bfloat16)
        dmask[:, s * 128:(s + 1) * 128] = bigdiag
        in_maps.append({
            "qfeat": np.ascontiguousarray(qfeats[f][:, s * QPC:(s + 1) * QPC]),
            "kfeat": kfeats[f],
            "diagmask": dmask,
            "ident": ident,
        })
    return in_maps, perms, pieces_per_half


def _postprocess(results, perms, pieces_per_half, atype):
    at = np.asarray(atype)
    pieces = [p for ph in pieces_per_half for p in ph]
    NP = len(pieces)
    min_rr2 = np.empty((NFRAMES, NLOC), np.float32)
    max_nnei = np.empty((NFRAMES, NTYPES), np.int64)
    for f in range(NFRAMES):
        counts = np.bincount(at[f], minlength=NTYPES)
        off = np.concatenate([[0], np.cumsum(counts)])
        ptype = [np.searchsorted(off, a, side="right") - 1 for a, _ in pieces]
        nnei_f = np.empty((NLOC, NTYPES), np.int64)
        for s in range(CPF):
            core = f * CPF + s
            r = results[core]
            mn = r["out_min"].reshape(128, NQT, 2).min(axis=2)
            min_sorted = mn.T.reshape(QPC)
            rows = perms[f][s * QPC:(s + 1) * QPC]
            min_rr2[f, rows] = min_sorted
            cnt = r["out_cnt"].reshape(128, NQT, NP)
            lt = np.zeros((128, NQT, NTYPES), np.float64)
            for j, (a, b) in enumerate(pieces):
                lt[:, :, ptype[j]] += (cnt[:, :, j] + (b - a)) * 0.5
            lt_q = np.transpose(lt, (1, 0, 2)).reshape(QPC, NTYPES)
            nnei_f[s * QPC:(s + 1) * QPC] = np.round(lt_q).astype(np.int64)
        max_nnei[f] = nnei_f.max(axis=0)
    return min_rr2, max_nnei.astype(np.int32)


def kernel(coord, atype):
    from concourse.bass_utils import run_bass_kernel_spmd

    in_maps, perms, pieces_per_half = _prep(coord, atype)
    key = tuple(tuple(p) for ph in pieces_per_half for p in ph)
    if key not in _CACHE:
        _CACHE[key] = _build(pieces_per_half)
    nc, NP = _CACHE[key]
    res = run_bass_kernel_spmd(nc, in_maps, list(range(NCORES)))
    return _postprocess(res.results, perms, pieces_per_half, atype)


# revision 5
# speedup vs baseline: 1.4708x; 1.4708x over previous
"""Trainium2 Bass kernel for NeighborStatOP (retrieval_knn).

Computes, for each frame and each local atom i:
  min_rr2[f, i]  = min_{j != i} |x_j - x_i|^2                      (f32)
  max_nnei[f, t] = max_i #{ j != i : |x_j - x_i|^2 < 6^2, type_j = t } (int32)

Strategy (8 NeuronCores, SPMD, one compiled program):
  - Atoms are spatially sorted host-side (Morton order), so each 128-query
    tile occupies a compact region.  For every tile the host builds a
    candidate key list: all atoms within RCUT(+eps) of the tile's bounding
    box.  Only ~1k of the 4096 keys survive -> ~3x less on-chip reduction
    work.  Candidates are grouped by atom type into four fixed-width
    segments (width = compile-time constant from the data, padded with
    far-away dummy atoms), so per-type counts use compile-time column
    ranges identical on every core.
  - Core c handles frame c//4, query tiles (c%4)*8 .. +8.
  - rr2 via tensor-engine matmul: centered per-component expansion
    qc^2 - 2 qc kc + kc^2 with every fp32 feature split into 3 bf16 pieces
    (bf16 x bf16 products are exact in fp32; PSUM accumulates fp32).
    K = 36 rows, full-rate bf16 matmuls.
  - Self-pair knockout: a second matmul per PSUM block accumulates
    rhs = per-tile diagmask (BIG=1e30 at each query's own column, else 0)
    through lhsT = identity, start=False.  Position is data, not code.
  - min: DVE tensor_reduce(min) per tile; counts: per type segment a
    ScalarE Sign pass (accum_out; count = (S + W)/2, self/pads give -1 so
    they drop out) or a DVE is_lt pass (count = S) - segment->engine
    assignment balances the two engines.
  - Host finalizes: inverse permutation, count formula, max; any query
    whose candidate min is >= RCUT^2 (nearest neighbour outside the
    candidate radius - does not happen for realistic densities) is
    recomputed exactly on host.
"""
import sys

sys.path.insert(0, "/opt/trn_rl_repo")

import numpy as np
import ml_dtypes

NFRAMES = 2
NLOC = 4096
NTYPES = 4
RCUT = 6.0
RCUT2 = 36.0
CENTER = 20.0
BIG = 1.0e30
PAD_COORD = 1000.0            # raw-coordinate value for padding atoms
NCORES = 8
CPF = 4                       # cores per frame
QPC = NLOC // CPF             # queries per core = 1024
NQT = QPC // 128              # query tiles per core = 8
NTILES = NLOC // 128          # query tiles per frame = 32
PAIRS = [(0, 0), (0, 1), (1, 0), (1, 1), (0, 2), (2, 0)]
K = 36                        # 12 split rows per component
SEG_ENGINE = ("A", "A", "D", "D")   # per-type-segment engine: ACT or DVE

_CACHE = {}


def _split3(x):
    p1 = x.astype(ml_dtypes.bfloat16)
    r1 = (x.astype(np.float64) - p1.astype(np.float64)).astype(np.float32)
    p2 = r1.astype(ml_dtypes.bfloat16)
    r2 = (r1.astype(np.float64) - p2.astype(np.float64)).astype(np.float32)
    p3 = r2.astype(ml_dtypes.bfloat16)
    return [p1, p2, p3]


def _features(coords):
    """coords: (n, 3) float64 centered.
    Returns (qfeat (K, n), kfeat (K, n)) as bfloat16 arrays."""
    n = len(coords)
    ones = np.ones(n, ml_dtypes.bfloat16)
    L, R = [], []
    for d in range(3):
        qc = coords[:, d]
        q2 = (qc * qc).astype(np.float32)
        q2p = _split3(q2)
        qp = _split3(qc.astype(np.float32))
        m2p = _split3((-2.0 * qc).astype(np.float32))
        for i in range(3):              # qc^2 pieces x 1
            L.append(q2p[i]); R.append(ones)
        for (ia, ib) in PAIRS:          # qc pieces x -2kc pieces
            L.append(qp[ia]); R.append(m2p[ib])
        for i in range(3):              # 1 x kc^2 pieces
            L.append(ones); R.append(q2p[i])
    return (np.stack(L).astype(ml_dtypes.bfloat16),
            np.stack(R).astype(ml_dtypes.bfloat16))


def _morton_perm(c):
    """c: (n, 3) raw coords in [0, 40]. 8x8x8 grid Morton order."""
    g = np.clip((c / 5.0).astype(np.int64), 0, 7)

    def spread(v):
        out = np.zeros_like(v)
        for b in range(3):
            out |= ((v >> b) & 1) << (3 * b)
        return out

    code = spread(g[:, 0]) | (spread(g[:, 1]) << 1) | (spread(g[:, 2]) << 2)
    return np.argsort(code, kind="stable")


def _build(wseg, repeat=1):
    """Build + lower the SPMD kernel. wseg: per-type segment width."""
    import concourse.bacc as bacc
    import concourse.tile as tile
    from concourse import mybir

    f32 = mybir.dt.float32
    bf16 = mybir.dt.bfloat16
    W = 4 * wseg
    WB = ((W + 511) // 512) * 512          # bank-padded width
    nbufs = max(2, (4096 // WB))

    nc = bacc.Bacc("TRN2", target_bir_lowering=False, debug=False,
                   num_devices=NCORES)
    qf = nc.dram_tensor("qfeat", [K, QPC], bf16, kind="ExternalInput").ap()
    kf = nc.dram_tensor("kfeat", [K, NQT * W], bf16,
                        kind="ExternalInput").ap()
    dm = nc.dram_tensor("diagmask", [128, NQT * W], bf16,
                        kind="ExternalInput").ap()
    idn = nc.dram_tensor("ident", [128, 128], bf16, kind="ExternalInput").ap()
    out_min = nc.dram_tensor("out_min", [128, NQT], f32,
                             kind="ExternalOutput").ap()
    out_cnt = nc.dram_tensor("out_cnt", [128, NQT * NTYPES], f32,
                             kind="ExternalOutput").ap()

    with tile.TileContext(nc) as tc:
        with (
            tc.tile_pool(name="singles", bufs=1) as singles,
            tc.tile_pool(name="psum", bufs=nbufs, space="PSUM") as psum_pool,
            tc.tile_pool(name="scratch", bufs=2) as scratch_pool,
        ):
            qsb = singles.tile([K, QPC], bf16)
            nc.sync.dma_start(out=qsb[:], in_=qf)
            ksb = singles.tile([K, NQT * W], bf16)
            nc.sync.dma_start(out=ksb[:], in_=kf)
            dmsb = singles.tile([128, NQT * W], bf16)
            nc.sync.dma_start(out=dmsb[:], in_=dm)
            idsb = singles.tile([128, 128], bf16)
            nc.sync.dma_start(out=idsb[:], in_=idn)
            bias36 = singles.tile([128, 1], f32)
            nc.vector.memset(bias36[:], RCUT2)
            min_sb = singles.tile([128, NQT], f32)
            cnt_sb = singles.tile([128, NQT * NTYPES], f32)

            def body(_iv=None):
                for t in range(NQT):
                    ps = psum_pool.tile([128, WB], f32, tag="ps")
                    for b0 in range(0, W, 512):
                        b1 = min(b0 + 512, W)
                        nc.tensor.matmul(
                            ps[:, b0:b1],
                            lhsT=qsb[:, t * 128:(t + 1) * 128],
                            rhs=ksb[:, t * W + b0:t * W + b1],
                            start=True, stop=False,
                        )
                        nc.tensor.matmul(
                            ps[:, b0:b1],
                            lhsT=idsb[:],
                            rhs=dmsb[:, t * W + b0:t * W + b1],
                            start=False, stop=True,
                            skip_group_check=True,
                        )
                    nc.vector.tensor_reduce(
                        out=min_sb[:, t:t + 1], in_=ps[:, 0:W],
                        axis=mybir.AxisListType.X, op=mybir.AluOpType.min,
                    )
                    sc = scratch_pool.tile([128, W], f32, tag="sc")
                    for s in range(NTYPES):
                        a, b_ = s * wseg, (s + 1) * wseg
                        ccol = t * NTYPES + s
                        if SEG_ENGINE[s] == "A":
                            nc.scalar.activation(
                                sc[:, a:b_], ps[:, a:b_],
                                mybir.ActivationFunctionType.Sign,
                                bias=bias36[:], scale=-1.0,
                                accum_out=cnt_sb[:, ccol:ccol + 1],
                            )
                        else:
                            nc.vector.tensor_scalar(
                                out=sc[:, a:b_], in0=ps[:, a:b_],
                                scalar1=RCUT2, scalar2=None,
                                op0=mybir.AluOpType.is_lt,
                                op1=mybir.AluOpType.add,
                                accum_out=cnt_sb[:, ccol:ccol + 1],
                            )

            if repeat == 1:
                body()
            else:
                with tc.For_i(0, repeat, 1) as iv:
                    body(iv)

            nc.sync.dma_start(out=out_min, in_=min_sb[:])
            nc.sync.dma_start(out=out_cnt, in_=cnt_sb[:])

    nc.compile()
    return nc


def _prep(coord, atype):
    """Host-side prep.

    Returns (in_maps, perms, wseg, straggler_info)."""
    c = np.asarray(coord, dtype=np.float32).reshape(NFRAMES, NLOC, 3)
    at = np.asarray(atype)

    perms, csorted, atsorted = [], [], []
    cand = [[None] * NTILES for _ in range(NFRAMES)]   # per frame/tile/type
    wmax = 0
    for f in range(NFRAMES):
        perm = _morton_perm(c[f])
        perms.append(perm)
        cs = c[f][perm]
        ats = at[f][perm]
        csorted.append(cs)
        atsorted.append(ats)
        cs64 = cs.astype(np.float64)
        for tg in range(NTILES):
            q = cs64[tg * 128:(tg + 1) * 128]
            lo = q.min(axis=0) - (RCUT + 0.01)
            hi = q.max(axis=0) + (RCUT + 0.01)
            inside = np.all((cs64 >= lo) & (cs64 <= hi), axis=1)
            idx = np.nonzero(inside)[0]
            bytype = [idx[ats[idx] == tt] for tt in range(NTYPES)]
            cand[f][tg] = bytype
            wmax = max(wmax, max(len(b) for b in bytype))
    wseg = max(64, ((wmax + 31) // 32) * 32)
    W = 4 * wseg

    ident = np.eye(128, dtype=ml_dtypes.bfloat16)

    in_maps = []
    for core in range(NCORES):
        f, s = core // CPF, core % CPF
        cs64 = csorted[f].astype(np.float64) - CENTER
        qfeat, kfeat_all = _features(cs64)
        # also features for one pad atom
        padf = _features(np.full((1, 3), PAD_COORD - CENTER))[1][:, 0]
        kfeat = np.empty((K, NQT * W), ml_dtypes.bfloat16)
        dmask = np.zeros((128, NQT * W), ml_dtypes.bfloat16)
        for t in range(NQT):
            tg = s * NQT + t
            col = t * W
            for tt in range(NTYPES):
                ids = cand[f][tg][tt]
                n = len(ids)
                seg = col + tt * wseg
                kfeat[:, seg:seg + n] = kfeat_all[:, ids]
                kfeat[:, seg + n:seg + wseg] = padf[:, None]
                # self positions: query row p (global atom tg*128+p) of type tt
                qrows = np.arange(128)
                qids = tg * 128 + qrows
                # find where each self atom sits in this segment
                pos = {int(a_): j for j, a_ in enumerate(ids)}
                for p in range(128):
                    j = pos.get(int(qids[p]))
                    if j is not None:
                        dmask[p, seg + j] = BIG
        in_maps.append({
            "qfeat": np.ascontiguousarray(
                qfeat[:, s * QPC:(s + 1) * QPC]),
            "kfeat": kfeat,
            "diagmask": dmask,
            "ident": ident,
        })
    return in_maps, perms, wseg


def _postprocess(results, perms, wseg, coord, atype):
    c = np.asarray(coord, dtype=np.float32).reshape(NFRAMES, NLOC, 3)
    at = np.asarray(atype)
    min_rr2 = np.empty((NFRAMES, NLOC), np.float32)
    max_nnei = np.empty((NFRAMES, NTYPES), np.int64)
    for f in range(NFRAMES):
        nnei_f = np.empty((NLOC, NTYPES), np.int64)
        mins_sorted = np.empty(NLOC, np.float32)
        for s in range(CPF):
            r = results[f * CPF + s]
            mn = r["out_min"]                      # (128, NQT)
            mins_sorted[s * QPC:(s + 1) * QPC] = mn.T.reshape(QPC)
            cnt = r["out_cnt"].reshape(128, NQT, NTYPES)
            lt = np.empty((128, NQT, NTYPES), np.float64)
            for tt in range(NTYPES):
                if SEG_ENGINE[tt] == "A":
                    lt[:, :, tt] = (cnt[:, :, tt] + wseg) * 0.5
                else:
                    lt[:, :, tt] = cnt[:, :, tt]
            nnei_f[s * QPC:(s + 1) * QPC] = np.round(
                np.transpose(lt, (1, 0, 2)).reshape(QPC, NTYPES)
            ).astype(np.int64)
        # stragglers: candidate min >= RCUT2 means the true nearest atom may
        # have been outside the candidate radius -> recompute exactly
        bad = np.nonzero(mins_sorted >= RCUT2)[0]
        if len(bad):
            cs = c[f][perms[f]].astype(np.float32)
            for i in bad:
                d = cs - cs[i]
                rr = (d[:, 0] * d[:, 0] + d[:, 1] * d[:, 1]
                      + d[:, 2] * d[:, 2]).astype(np.float32)
                rr[i] = np.inf
                mins_sorted[i] = rr.min()
        min_rr2[f, perms[f]] = mins_sorted
        max_nnei[f] = nnei_f.max(axis=0)
    return min_rr2, max_nnei.astype(np.int32)


def kernel(coord, atype):
    from concourse.bass_utils import run_bass_kernel_spmd

    in_maps, perms, wseg = _prep(coord, atype)
    if wseg not in _CACHE:
        _CACHE[wseg] = _build(wseg)
    nc = _CACHE[wseg]
    res = run_bass_kernel_spmd(nc, in_maps, list(range(NCORES)))
    return _postprocess(res.results, perms, wseg, coord, atype)


# revision 6
# speedup vs baseline: 2.5438x; 1.7296x over previous
"""Trainium2 Bass kernel for NeighborStatOP (retrieval_knn).

Computes, for each frame and each local atom i:
  min_rr2[f, i]  = min_{j != i} |x_j - x_i|^2                      (f32)
  max_nnei[f, t] = max_i #{ j != i : |x_j - x_i|^2 < 6^2, type_j = t } (int32)

Strategy (8 NeuronCores, SPMD, one compiled program):
  - Atoms are sorted host-side by a kd-tree (median splits), so each
    128-query tile occupies a compact box.  For every tile the host builds
    a candidate key list: all atoms within RCUT(+eps) of the tile's box
    (exact point-to-box distance).  Only ~400-1300 of the 4096 keys
    survive -> ~3x less on-chip reduction work.  Candidates are grouped by
    atom type into four fixed-width segments per processing slot; each
    core processes its tiles largest-first and slot widths are the
    per-slot maxima over cores, so a single sparse-region outlier tile
    doesn't inflate every slot.  Padding uses far-away dummy atoms.
  - Core c handles frame c//4, query tiles (c%4)*8 .. +8 (kd order).
  - rr2 via tensor-engine matmul: centered per-component expansion
    qc^2 - 2 qc kc + kc^2 with every fp32 feature split into 3 bf16 pieces
    (bf16 x bf16 products are exact in fp32; PSUM accumulates fp32).
    K = 36 rows, full-rate bf16 matmuls.
  - Self-pair knockout: within each type segment the tile's own atoms are
    listed first, so all BIG (1e30) entries live in a 128-wide window at
    each segment start; a small extra matmul per window accumulates the
    per-tile diagmask (BIG at each query's own column) through
    lhsT = identity, start=False.  Position is data, not code.
  - min: DVE tensor_reduce(min) per slot; counts: per type segment a
    ScalarE Sign pass (accum_out; count = (S + W)/2; self/pads give -1 and
    drop out) or a DVE is_lt pass (count = S) - the segment->engine map
    balances ACT and DVE.
  - Host finalizes: count formula, inverse permutations, max; any query
    whose candidate min is >= RCUT^2 (nearest neighbour outside the
    candidate radius; does not occur at realistic densities) is recomputed
    exactly on host.
"""
import sys

sys.path.insert(0, "/opt/trn_rl_repo")

import numpy as np
import ml_dtypes

NFRAMES = 2
NLOC = 4096
NTYPES = 4
RCUT = 6.0
RCUT2 = 36.0
CENTER = 20.0
BIG = 1.0e30
PAD_COORD = 1000.0
NCORES = 8
CPF = 4                       # cores per frame
QPC = NLOC // CPF             # queries per core = 1024
NQT = QPC // 128              # query tiles (slots) per core = 8
NTILES = NLOC // 128          # query tiles per frame = 32
PAIRS = [(0, 0), (0, 1), (1, 0), (1, 1), (0, 2), (2, 0)]
K = 36                        # 12 split rows per component
SEG_ENGINE = ("A", "A", "A", "D")   # per-type-segment engine: ACT or DVE

_CACHE = {}


def _split3(x):
    p1 = x.astype(ml_dtypes.bfloat16)
    r1 = (x.astype(np.float64) - p1.astype(np.float64)).astype(np.float32)
    p2 = r1.astype(ml_dtypes.bfloat16)
    r2 = (r1.astype(np.float64) - p2.astype(np.float64)).astype(np.float32)
    p3 = r2.astype(ml_dtypes.bfloat16)
    return [p1, p2, p3]


def _features(coords):
    """coords: (n, 3) float64 centered.
    Returns (qfeat (K, n), kfeat (K, n)) as bfloat16 arrays."""
    n = len(coords)
    ones = np.ones(n, ml_dtypes.bfloat16)
    L, R = [], []
    for d in range(3):
        qc = coords[:, d]
        q2 = (qc * qc).astype(np.float32)
        q2p = _split3(q2)
        qp = _split3(qc.astype(np.float32))
        m2p = _split3((-2.0 * qc).astype(np.float32))
        for i in range(3):              # qc^2 pieces x 1
            L.append(q2p[i]); R.append(ones)
        for (ia, ib) in PAIRS:          # qc pieces x -2kc pieces
            L.append(qp[ia]); R.append(m2p[ib])
        for i in range(3):              # 1 x kc^2 pieces
            L.append(ones); R.append(q2p[i])
    return (np.stack(L).astype(ml_dtypes.bfloat16),
            np.stack(R).astype(ml_dtypes.bfloat16))


def _kd_perm(c):
    """c: (n, 3) raw coords. Recursive median split into 128-atom leaves."""
    def split(ids):
        if len(ids) <= 128:
            return [ids]
        spread = c[ids].max(0) - c[ids].min(0)
        ax = int(np.argmax(spread))
        order = ids[np.argsort(c[ids, ax], kind="stable")]
        h = len(ids) // 2
        return split(order[:h]) + split(order[h:])
    return np.concatenate(split(np.arange(len(c))))


def _build(wslots, repeat=1):
    """Build + lower the SPMD kernel. wslots: per-slot segment widths."""
    import concourse.bacc as bacc
    import concourse.tile as tile
    from concourse import mybir

    f32 = mybir.dt.float32
    bf16 = mybir.dt.bfloat16
    Ws = [4 * w for w in wslots]
    col0 = np.concatenate([[0], np.cumsum(Ws)])      # kfeat col offset per slot
    WTOT = int(col0[-1])
    WBMAX = ((max(Ws) + 511) // 512) * 512

    nc = bacc.Bacc("TRN2", target_bir_lowering=False, debug=False,
                   num_devices=NCORES)
    qf = nc.dram_tensor("qfeat", [K, QPC], bf16, kind="ExternalInput").ap()
    kf = nc.dram_tensor("kfeat", [K, WTOT], bf16, kind="ExternalInput").ap()
    dm = nc.dram_tensor("diagmask", [128, NQT * NTYPES * 128], bf16,
                        kind="ExternalInput").ap()
    idn = nc.dram_tensor("ident", [128, 128], bf16, kind="ExternalInput").ap()
    out_min = nc.dram_tensor("out_min", [128, NQT], f32,
                             kind="ExternalOutput").ap()
    out_cnt = nc.dram_tensor("out_cnt", [128, NQT * NTYPES], f32,
                             kind="ExternalOutput").ap()

    with tile.TileContext(nc) as tc:
        with (
            tc.tile_pool(name="singles", bufs=1) as singles,
            tc.tile_pool(name="psum", bufs=2, space="PSUM") as psum_pool,
            tc.tile_pool(name="scratch", bufs=2) as scratch_pool,
        ):
            qsb = singles.tile([K, QPC], bf16)
            nc.sync.dma_start(out=qsb[:], in_=qf)
            ksb = singles.tile([K, WTOT], bf16)
            dmsb = singles.tile([128, NQT * NTYPES * 128], bf16)
            for t in range(NQT):       # per-slot DMAs so slot 0 starts early
                a, b = int(col0[t]), int(col0[t + 1])
                nc.sync.dma_start(out=ksb[:, a:b], in_=kf[:, a:b])
                da, db = t * NTYPES * 128, (t + 1) * NTYPES * 128
                nc.sync.dma_start(out=dmsb[:, da:db], in_=dm[:, da:db])
            idsb = singles.tile([128, 128], bf16)
            nc.sync.dma_start(out=idsb[:], in_=idn)
            bias36 = singles.tile([128, 1], f32)
            nc.vector.memset(bias36[:], RCUT2)
            min_sb = singles.tile([128, NQT], f32)
            cnt_sb = singles.tile([128, NQT * NTYPES], f32)

            def body(_iv=None):
                for t in range(NQT):
                    w = wslots[t]
                    W = 4 * w
                    k0 = int(col0[t])
                    ps = psum_pool.tile([128, WBMAX], f32, tag="ps")
                    # diag windows: [tau*w, tau*w + min(128, w)) split at banks
                    by_bank = {}
                    for tau in range(NTYPES):
                        a = tau * w
                        b = a + min(128, w)
                        while a < b:
                            e = min(b, ((a // 512) + 1) * 512)
                            by_bank.setdefault(a // 512, []).append(
                                (a, e, tau, a - tau * w))
                            a = e
                    for b0 in range(0, W, 512):
                        b1 = min(b0 + 512, W)
                        nc.tensor.matmul(
                            ps[:, b0:b1],
                            lhsT=qsb[:, t * 128:(t + 1) * 128],
                            rhs=ksb[:, k0 + b0:k0 + b1],
                            start=True, stop=(b0 // 512) not in by_bank,
                        )
                    for bank in sorted(by_bank):
                        subs = by_bank[bank]
                        for i, (a, e, tau, woff) in enumerate(subs):
                            dcol = (t * NTYPES + tau) * 128 + woff
                            nc.tensor.matmul(
                                ps[:, a:e],
                                lhsT=idsb[:],
                                rhs=dmsb[:, dcol:dcol + (e - a)],
                                start=False, stop=(i == len(subs) - 1),
                                skip_group_check=True,
                            )
                    nc.vector.tensor_reduce(
                        out=min_sb[:, t:t + 1], in_=ps[:, 0:W],
                        axis=mybir.AxisListType.X, op=mybir.AluOpType.min,
                    )
                    sc = scratch_pool.tile([128, WBMAX], f32, tag="sc")
                    for s in range(NTYPES):
                        a, b_ = s * w, (s + 1) * w
                        ccol = t * NTYPES + s
                        if SEG_ENGINE[s] == "A":
                            nc.scalar.activation(
                                sc[:, a:b_], ps[:, a:b_],
                                mybir.ActivationFunctionType.Sign,
                                bias=bias36[:], scale=-1.0,
                                accum_out=cnt_sb[:, ccol:ccol + 1],
                            )
                        else:
                            nc.vector.tensor_scalar(
                                out=sc[:, a:b_], in0=ps[:, a:b_],
                                scalar1=RCUT2, scalar2=None,
                                op0=mybir.AluOpType.is_lt,
                                op1=mybir.AluOpType.add,
                                accum_out=cnt_sb[:, ccol:ccol + 1],
                            )

            if repeat == 1:
                body()
            else:
                with tc.For_i(0, repeat, 1) as iv:
                    body(iv)

            nc.sync.dma_start(out=out_min, in_=min_sb[:])
            nc.sync.dma_start(out=out_cnt, in_=cnt_sb[:])

    nc.compile()
    return nc


def _prep(coord, atype):
    """Host-side prep.

    Returns (in_maps, perms, tile_orders, wslots)."""
    c = np.asarray(coord, dtype=np.float32).reshape(NFRAMES, NLOC, 3)
    at = np.asarray(atype)

    perms, atsorted, csorted = [], [], []
    cand = [[None] * NTILES for _ in range(NFRAMES)]
    for f in range(NFRAMES):
        perm = _kd_perm(c[f])
        perms.append(perm)
        cs = c[f][perm]
        ats = at[f][perm]
        csorted.append(cs)
        atsorted.append(ats)
        cs64 = cs.astype(np.float64)
        for tg in range(NTILES):
            q = cs64[tg * 128:(tg + 1) * 128]
            lo, hi = q.min(0), q.max(0)
            d = np.maximum(np.maximum(lo - cs64, cs64 - hi), 0.0)
            idx = np.nonzero((d * d).sum(1) <= (RCUT + 0.01) ** 2)[0]
            # tile's own atoms first within each type (diagmask window)
            own = (idx >= tg * 128) & (idx < (tg + 1) * 128)
            bytype = []
            for tt in range(NTYPES):
                sel = idx[ats[idx] == tt]
                o = sel[(sel >= tg * 128) & (sel < (tg + 1) * 128)]
                rest = sel[(sel < tg * 128) | (sel >= (tg + 1) * 128)]
                bytype.append(np.concatenate([o, rest]))
            cand[f][tg] = bytype

    # processing order: per core, tiles sorted by type-max width descending
    tile_orders = []       # per core: list of global tile ids in slot order
    for core in range(NCORES):
        f, s = core // CPF, core % CPF
        tiles = list(range(s * NQT, (s + 1) * NQT))
        tiles.sort(key=lambda tg: -max(len(b) for b in cand[f][tg]))
        tile_orders.append(tiles)
    wslots = []
    for t in range(NQT):
        wmax = 0
        for core in range(NCORES):
            f = core // CPF
            tg = tile_orders[core][t]
            wmax = max(wmax, max(len(b) for b in cand[f][tg]))
        wslots.append(max(128, ((wmax + 31) // 32) * 32))
    Ws = [4 * w for w in wslots]
    col0 = np.concatenate([[0], np.cumsum(Ws)]).astype(int)
    WTOT = int(col0[-1])

    ident = np.eye(128, dtype=ml_dtypes.bfloat16)

    in_maps = []
    for core in range(NCORES):
        f, s = core // CPF, core % CPF
        cs64 = csorted[f].astype(np.float64) - CENTER
        qfeat_all, kfeat_all = _features(cs64)
        padf = _features(np.full((1, 3), PAD_COORD - CENTER))[1][:, 0]
        kfeat = np.empty((K, WTOT), ml_dtypes.bfloat16)
        dmask = np.zeros((128, NQT * NTYPES * 128), ml_dtypes.bfloat16)
        qfeat = np.empty((K, QPC), ml_dtypes.bfloat16)
        for t in range(NQT):
            tg = tile_orders[core][t]
            w = wslots[t]
            qfeat[:, t * 128:(t + 1) * 128] = \
                qfeat_all[:, tg * 128:(tg + 1) * 128]
            for tt in range(NTYPES):
                ids = cand[f][tg][tt]
                n = len(ids)
                seg = int(col0[t]) + tt * w
                kfeat[:, seg:seg + n] = kfeat_all[:, ids]
                kfeat[:, seg + n:seg + w] = padf[:, None]
                # own atoms sit at positions 0..m-1 of this segment
                for j, a_ in enumerate(ids):
                    if tg * 128 <= a_ < (tg + 1) * 128:
                        p = int(a_) - tg * 128
                        dmask[p, (t * NTYPES + tt) * 128 + j] = BIG
        in_maps.append({
            "qfeat": qfeat,
            "kfeat": kfeat,
            "diagmask": dmask,
            "ident": ident,
        })
    return in_maps, perms, tile_orders, wslots


def _postprocess(results, perms, tile_orders, wslots, coord, atype):
    c = np.asarray(coord, dtype=np.float32).reshape(NFRAMES, NLOC, 3)
    min_rr2 = np.empty((NFRAMES, NLOC), np.float32)
    max_nnei = np.empty((NFRAMES, NTYPES), np.int64)
    nnei_max = np.zeros((NFRAMES, NTYPES), np.int64)
    mins_sorted = [np.empty(NLOC, np.float32) for _ in range(NFRAMES)]
    for core in range(NCORES):
        f, s = core // CPF, core % CPF
        r = results[core]
        mn = r["out_min"]                      # (128, NQT)
        cnt = r["out_cnt"].reshape(128, NQT, NTYPES)
        for t in range(NQT):
            tg = tile_orders[core][t]
            mins_sorted[f][tg * 128:(tg + 1) * 128] = mn[:, t]
            for tt in range(NTYPES):
                if SEG_ENGINE[tt] == "A":
                    lt = (cnt[:, t, tt] + wslots[t]) * 0.5
                else:
                    lt = cnt[:, t, tt]
                nnei_max[f, tt] = max(nnei_max[f, tt],
                                      int(np.round(lt.max())))
    for f in range(NFRAMES):
        ms = mins_sorted[f]
        bad = np.nonzero(ms >= RCUT2)[0]
        if len(bad):
            cs = c[f][perms[f]].astype(np.float32)
            for i in bad:
                d = cs - cs[i]
                rr = (d[:, 0] * d[:, 0] + d[:, 1] * d[:, 1]
                      + d[:, 2] * d[:, 2]).astype(np.float32)
                rr[i] = np.inf
                ms[i] = rr.min()
        min_rr2[f, perms[f]] = ms
        max_nnei[f] = nnei_max[f]
    return min_rr2, max_nnei.astype(np.int32)


def kernel(coord, atype):
    from concourse.bass_utils import run_bass_kernel_spmd

    in_maps, perms, tile_orders, wslots = _prep(coord, atype)
    key = tuple(wslots)
    if key not in _CACHE:
        _CACHE[key] = _build(wslots)
    nc = _CACHE[key]
    res = run_bass_kernel_spmd(nc, in_maps, list(range(NCORES)))
    return _postprocess(res.results, perms, tile_orders, wslots,
                        coord, atype)


# revision 9
# speedup vs baseline: 2.8684x; 1.1276x over previous
"""Trainium2 Bass kernel for NeighborStatOP (retrieval_knn).

Computes, for each frame and each local atom i:
  min_rr2[f, i]  = min_{j != i} |x_j - x_i|^2                      (f32)
  max_nnei[f, t] = max_i #{ j != i : |x_j - x_i|^2 < 6^2, type_j = t } (int32)

Strategy (8 NeuronCores, SPMD, one compiled program):
  - Atoms are sorted host-side by a kd-tree (median splits), so each
    128-query tile occupies a compact box.  For every tile the host builds
    a candidate key list: all atoms within RCUT(+eps) of the tile's box
    (exact point-to-box distance).  Only ~400-1300 of the 4096 keys
    survive -> ~3x less on-chip reduction work.  Candidates are grouped by
    atom type into four fixed-width segments per processing slot; each
    core processes its tiles largest-first and slot widths are the
    per-slot maxima over cores, so a single sparse-region outlier tile
    doesn't inflate every slot.  Padding uses far-away dummy atoms.
  - Core c handles frame c//4, query tiles (c%4)*8 .. +8 (kd order).
  - rr2 via tensor-engine matmul: centered per-component expansion
    qc^2 - 2 qc kc + kc^2 with every fp32 feature split into 3 bf16 pieces
    (bf16 x bf16 products are exact in fp32; PSUM accumulates fp32).
    K = 36 rows, full-rate bf16 matmuls.
  - Self-pair knockout: within each type segment the tile's own atoms are
    listed first, so all BIG (1e30) entries live in a 128-wide window at
    each segment start; a small extra matmul per window accumulates the
    per-tile diagmask (BIG at each query's own column) through
    lhsT = identity, start=False.  Position is data, not code.
  - min: DVE tensor_reduce(min) per slot; counts: per type segment a
    ScalarE Sign pass (accum_out; count = (S + W)/2; self/pads give -1 and
    drop out) or a DVE is_lt pass (count = S) - the segment->engine map
    balances ACT and DVE.
  - Host finalizes: count formula, inverse permutations, max; any query
    whose candidate min is >= RCUT^2 (nearest neighbour outside the
    candidate radius; does not occur at realistic densities) is recomputed
    exactly on host.
"""
import sys

sys.path.insert(0, "/opt/trn_rl_repo")

import numpy as np
import ml_dtypes

NFRAMES = 2
NLOC = 4096
NTYPES = 4
RCUT = 6.0
RCUT2 = 36.0
CENTER = 20.0
BIG = 1.0e30
PAD_COORD = 1000.0
NCORES = 8
CPF = 4                       # cores per frame
QPC = NLOC // CPF             # queries per core = 1024
NQT = QPC // 128              # query tiles (slots) per core = 8
NTILES = NLOC // 128          # query tiles per frame = 32
PAIRS = [(0, 0), (0, 1), (1, 0), (1, 1), (0, 2), (2, 0)]
K = 36                        # 12 split rows per component


def seg_engine(t, tau):
    """Engine for the count pass of slot t, type-segment tau (balance)."""
    return "D" if tau == 3 or (tau == 2 and t >= 4) else "A"

_CACHE = {}


def _split3(x):
    p1 = x.astype(ml_dtypes.bfloat16)
    r1 = (x.astype(np.float64) - p1.astype(np.float64)).astype(np.float32)
    p2 = r1.astype(ml_dtypes.bfloat16)
    r2 = (r1.astype(np.float64) - p2.astype(np.float64)).astype(np.float32)
    p3 = r2.astype(ml_dtypes.bfloat16)
    return [p1, p2, p3]


def _features(coords):
    """coords: (n, 3) float64 centered.
    Returns (qfeat (K, n), kfeat (K, n)) as bfloat16 arrays."""
    n = len(coords)
    ones = np.ones(n, ml_dtypes.bfloat16)
    L, R = [], []
    for d in range(3):
        qc = coords[:, d]
        q2 = (qc * qc).astype(np.float32)
        q2p = _split3(q2)
        qp = _split3(qc.astype(np.float32))
        m2p = _split3((-2.0 * qc).astype(np.float32))
        for i in range(3):              # qc^2 pieces x 1
            L.append(q2p[i]); R.append(ones)
        for (ia, ib) in PAIRS:          # qc pieces x -2kc pieces
            L.append(qp[ia]); R.append(m2p[ib])
        for i in range(3):              # 1 x kc^2 pieces
            L.append(ones); R.append(q2p[i])
    return (np.stack(L).astype(ml_dtypes.bfloat16),
            np.stack(R).astype(ml_dtypes.bfloat16))


def _kd_perm(c):
    """c: (n, 3) raw coords. Recursive median split into 128-atom leaves."""
    def split(ids):
        if len(ids) <= 128:
            return [ids]
        spread = c[ids].max(0) - c[ids].min(0)
        ax = int(np.argmax(spread))
        order = ids[np.argsort(c[ids, ax], kind="stable")]
        h = len(ids) // 2
        return split(order[:h]) + split(order[h:])
    return np.concatenate(split(np.arange(len(c))))


def _build(wslots, repeat=1):
    """Build + lower the SPMD kernel. wslots: per-slot segment widths."""
    import concourse.bacc as bacc
    import concourse.tile as tile
    from concourse import mybir

    f32 = mybir.dt.float32
    bf16 = mybir.dt.bfloat16
    Ws = [4 * w for w in wslots]
    col0 = np.concatenate([[0], np.cumsum(Ws)])      # kfeat col offset per slot
    WTOT = int(col0[-1])
    WBMAX = ((max(Ws) + 511) // 512) * 512

    nc = bacc.Bacc("TRN2", target_bir_lowering=False, debug=False,
                   num_devices=NCORES)
    qf = nc.dram_tensor("qfeat", [K, QPC], bf16, kind="ExternalInput").ap()
    kf = nc.dram_tensor("kfeat", [K, WTOT], bf16, kind="ExternalInput").ap()
    dm = nc.dram_tensor("diagmask", [128, NQT * NTYPES * 128], bf16,
                        kind="ExternalInput").ap()
    idn = nc.dram_tensor("ident", [128, 128], bf16, kind="ExternalInput").ap()
    out_min = nc.dram_tensor("out_min", [128, NQT], f32,
                             kind="ExternalOutput").ap()
    out_cnt = nc.dram_tensor("out_cnt", [128, NQT * NTYPES], f32,
                             kind="ExternalOutput").ap()

    wide = [t for t in range(NQT) if 4 * wslots[t] > 512]
    narrow = [t for t in range(NQT) if 4 * wslots[t] <= 512]
    WBWIDE = ((max((4 * wslots[t] for t in wide), default=512) + 511)
              // 512) * 512

    with tile.TileContext(nc) as tc:
        with (
            tc.tile_pool(name="singles", bufs=1) as singles,
            tc.tile_pool(name="sc_act", bufs=3) as sc_act_pool,
            tc.tile_pool(name="sc_dve", bufs=2) as sc_dve_pool,
        ):
            qsb = singles.tile([K, QPC], bf16)
            nc.sync.dma_start(out=qsb[:], in_=qf)
            ksb = singles.tile([K, WTOT], bf16)
            dmsb = singles.tile([128, NQT * NTYPES * 128], bf16)
            for t in range(NQT):       # per-slot DMAs so slot 0 starts early
                a, b = int(col0[t]), int(col0[t + 1])
                nc.sync.dma_start(out=ksb[:, a:b], in_=kf[:, a:b])
                da, db = t * NTYPES * 128, (t + 1) * NTYPES * 128
                nc.sync.dma_start(out=dmsb[:, da:db], in_=dm[:, da:db])
            idsb = singles.tile([128, 128], bf16)
            nc.sync.dma_start(out=idsb[:], in_=idn)
            bias36 = singles.tile([128, 1], f32)
            nc.vector.memset(bias36[:], RCUT2)
            min_sb = singles.tile([128, NQT], f32)
            cnt_sb = singles.tile([128, NQT * NTYPES], f32)

            def do_slot(t, psum_pool, wb):
                w = wslots[t]
                W = 4 * w
                k0 = int(col0[t])
                ps = psum_pool.tile([128, wb], f32, tag="ps")
                # diag windows: [tau*w, tau*w + min(128, w)) split at banks
                by_bank = {}
                for tau in range(NTYPES):
                    a = tau * w
                    b = a + min(128, w)
                    while a < b:
                        e = min(b, ((a // 512) + 1) * 512)
                        by_bank.setdefault(a // 512, []).append(
                            (a, e, tau, a - tau * w))
                        a = e
                for b0 in range(0, W, 512):
                    b1 = min(b0 + 512, W)
                    nc.tensor.matmul(
                        ps[:, b0:b1],
                        lhsT=qsb[:, t * 128:(t + 1) * 128],
                        rhs=ksb[:, k0 + b0:k0 + b1],
                        start=True, stop=(b0 // 512) not in by_bank,
                    )
                for bank in sorted(by_bank):
                    subs = by_bank[bank]
                    for i, (a, e, tau, woff) in enumerate(subs):
                        dcol = (t * NTYPES + tau) * 128 + woff
                        nc.tensor.matmul(
                            ps[:, a:e],
                            lhsT=idsb[:],
                            rhs=dmsb[:, dcol:dcol + (e - a)],
                            start=False, stop=(i == len(subs) - 1),
                            skip_group_check=True,
                        )
                nc.vector.tensor_reduce(
                    out=min_sb[:, t:t + 1], in_=ps[:, 0:W],
                    axis=mybir.AxisListType.X, op=mybir.AluOpType.min,
                )
                for s in range(NTYPES):
                    a, b_ = s * w, (s + 1) * w
                    ccol = t * NTYPES + s
                    if seg_engine(t, s) == "A":
                        sc = sc_act_pool.tile([128, WBWIDE], f32, tag="sa")
                        nc.scalar.activation(
                            sc[:, a:b_], ps[:, a:b_],
                            mybir.ActivationFunctionType.Sign,
                            bias=bias36[:], scale=-1.0,
                            accum_out=cnt_sb[:, ccol:ccol + 1],
                        )
                    else:
                        sc = sc_dve_pool.tile([128, WBWIDE], f32, tag="sd")
                        nc.vector.tensor_scalar(
                            out=sc[:, a:b_], in0=ps[:, a:b_],
                            scalar1=RCUT2, scalar2=None,
                            op0=mybir.AluOpType.is_lt,
                            op1=mybir.AluOpType.add,
                            accum_out=cnt_sb[:, ccol:ccol + 1],
                        )

            def body(_iv=None):
                with tc.tile_pool(name="psA", bufs=2, space="PSUM") as pA:
                    for t in wide:
                        do_slot(t, pA, WBWIDE)
                with tc.tile_pool(name="psB", bufs=max(2, len(narrow)),
                                  space="PSUM") as pB:
                    for t in narrow:
                        do_slot(t, pB, 512)

            if repeat == 1:
                body()
            else:
                with tc.For_i(0, repeat, 1) as iv:
                    body(iv)

            nc.sync.dma_start(out=out_min, in_=min_sb[:])
            nc.sync.dma_start(out=out_cnt, in_=cnt_sb[:])

    nc.compile()
    return nc


def _prep(coord, atype):
    """Host-side prep.

    Returns (in_maps, perms, tile_orders, wslots)."""
    c = np.asarray(coord, dtype=np.float32).reshape(NFRAMES, NLOC, 3)
    at = np.asarray(atype)

    perms, atsorted, csorted = [], [], []
    cand = [[None] * NTILES for _ in range(NFRAMES)]
    for f in range(NFRAMES):
        perm = _kd_perm(c[f])
        perms.append(perm)
        cs = c[f][perm]
        ats = at[f][perm]
        csorted.append(cs)
        atsorted.append(ats)
        cs64 = cs.astype(np.float64)
        for tg in range(NTILES):
            q = cs64[tg * 128:(tg + 1) * 128]
            lo, hi = q.min(0), q.max(0)
            d = np.maximum(np.maximum(lo - cs64, cs64 - hi), 0.0)
            idx = np.nonzero((d * d).sum(1) <= (RCUT + 0.01) ** 2)[0]
            # tile's own atoms first within each type (diagmask window)
            own = (idx >= tg * 128) & (idx < (tg + 1) * 128)
            bytype = []
            for tt in range(NTYPES):
                sel = idx[ats[idx] == tt]
                o = sel[(sel >= tg * 128) & (sel < (tg + 1) * 128)]
                rest = sel[(sel < tg * 128) | (sel >= (tg + 1) * 128)]
                bytype.append(np.concatenate([o, rest]))
            cand[f][tg] = bytype

    # processing order: per core, tiles sorted by type-max width descending
    tile_orders = []       # per core: list of global tile ids in slot order
    for core in range(NCORES):
        f, s = core // CPF, core % CPF
        tiles = list(range(s * NQT, (s + 1) * NQT))
        tiles.sort(key=lambda tg: -max(len(b) for b in cand[f][tg]))
        tile_orders.append(tiles)
    wslots = []
    for t in range(NQT):
        wmax = 0
        for core in range(NCORES):
            f = core // CPF
            tg = tile_orders[core][t]
            wmax = max(wmax, max(len(b) for b in cand[f][tg]))
        wslots.append(max(128, ((wmax + 31) // 32) * 32))
    Ws = [4 * w for w in wslots]
    col0 = np.concatenate([[0], np.cumsum(Ws)]).astype(int)
    WTOT = int(col0[-1])

    ident = np.eye(128, dtype=ml_dtypes.bfloat16)

    in_maps = []
    for core in range(NCORES):
        f, s = core // CPF, core % CPF
        cs64 = csorted[f].astype(np.float64) - CENTER
        qfeat_all, kfeat_all = _features(cs64)
        padf = _features(np.full((1, 3), PAD_COORD - CENTER))[1][:, 0]
        kfeat = np.empty((K, WTOT), ml_dtypes.bfloat16)
        dmask = np.zeros((128, NQT * NTYPES * 128), ml_dtypes.bfloat16)
        qfeat = np.empty((K, QPC), ml_dtypes.bfloat16)
        for t in range(NQT):
            tg = tile_orders[core][t]
            w = wslots[t]
            qfeat[:, t * 128:(t + 1) * 128] = \
                qfeat_all[:, tg * 128:(tg + 1) * 128]
            for tt in range(NTYPES):
                ids = cand[f][tg][tt]
                n = len(ids)
                seg = int(col0[t]) + tt * w
                kfeat[:, seg:seg + n] = kfeat_all[:, ids]
                kfeat[:, seg + n:seg + w] = padf[:, None]
                # own atoms sit at positions 0..m-1 of this segment
                for j, a_ in enumerate(ids):
                    if tg * 128 <= a_ < (tg + 1) * 128:
                        p = int(a_) - tg * 128
                        dmask[p, (t * NTYPES + tt) * 128 + j] = BIG
        in_maps.append({
            "qfeat": qfeat,
            "kfeat": kfeat,
            "diagmask": dmask,
            "ident": ident,
        })
    return in_maps, perms, tile_orders, wslots


def _postprocess(results, perms, tile_orders, wslots, coord, atype):
    c = np.asarray(coord, dtype=np.float32).reshape(NFRAMES, NLOC, 3)
    min_rr2 = np.empty((NFRAMES, NLOC), np.float32)
    max_nnei = np.empty((NFRAMES, NTYPES), np.int64)
    nnei_max = np.zeros((NFRAMES, NTYPES), np.int64)
    mins_sorted = [np.empty(NLOC, np.float32) for _ in range(NFRAMES)]
    for core in range(NCORES):
        f, s = core // CPF, core % CPF
        r = results[core]
        mn = r["out_min"]                      # (128, NQT)
        cnt = r["out_cnt"].reshape(128, NQT, NTYPES)
        for t in range(NQT):
            tg = tile_orders[core][t]
            mins_sorted[f][tg * 128:(tg + 1) * 128] = mn[:, t]
            for tt in range(NTYPES):
                if seg_engine(t, tt) == "A":
                    lt = (cnt[:, t, tt] + wslots[t]) * 0.5
                else:
                    lt = cnt[:, t, tt]
                nnei_max[f, tt] = max(nnei_max[f, tt],
                                      int(np.round(lt.max())))
    for f in range(NFRAMES):
        ms = mins_sorted[f]
        bad = np.nonzero(ms >= RCUT2)[0]
        if len(bad):
            cs = c[f][perms[f]].astype(np.float32)
            for i in bad:
                d = cs - cs[i]
                rr = (d[:, 0] * d[:, 0] + d[:, 1] * d[:, 1]
                      + d[:, 2] * d[:, 2]).astype(np.float32)
                rr[i] = np.inf
                ms[i] = rr.min()
        min_rr2[f, perms[f]] = ms
        max_nnei[f] = nnei_max[f]
    return min_rr2, max_nnei.astype(np.int32)


def kernel(coord, atype):
    from concourse.bass_utils import run_bass_kernel_spmd

    in_maps, perms, tile_orders, wslots = _prep(coord, atype)
    key = tuple(wslots)
    if key not in _CACHE:
        _CACHE[key] = _build(wslots)
    nc = _CACHE[key]
    res = run_bass_kernel_spmd(nc, in_maps, list(range(NCORES)))
    return _postprocess(res.results, perms, tile_orders, wslots,
                        coord, atype)


# revision 11
# speedup vs baseline: 3.0402x; 1.0599x over previous
"""Trainium2 Bass kernel for NeighborStatOP (retrieval_knn).

Computes, for each frame and each local atom i:
  min_rr2[f, i]  = min_{j != i} |x_j - x_i|^2                      (f32)
  max_nnei[f, t] = max_i #{ j != i : |x_j - x_i|^2 < 6^2, type_j = t } (int32)

Strategy (8 NeuronCores, SPMD, one compiled program):
  - Atoms are sorted host-side by a kd-tree (median splits), so each
    128-query tile occupies a compact box.  For every tile the host builds
    a candidate key list: all atoms within RCUT(+eps) of the tile's box
    (exact point-to-box distance).  Only ~400-1300 of the 4096 keys
    survive -> ~3x less on-chip reduction work.  Candidates are grouped by
    atom type into four fixed-width segments per processing slot; each
    core processes its tiles largest-first and slot widths are the
    per-slot maxima over cores, so a single sparse-region outlier tile
    doesn't inflate every slot.  Padding uses far-away dummy atoms.
  - Core c handles frame c//4, query tiles (c%4)*8 .. +8 (kd order).
  - rr2 via tensor-engine matmul: centered per-component expansion
    qc^2 - 2 qc kc + kc^2 with every fp32 feature split into 3 bf16 pieces
    (bf16 x bf16 products are exact in fp32; PSUM accumulates fp32).
    K = 36 rows, full-rate bf16 matmuls.
  - Self-pair knockout: within each type segment the tile's own atoms are
    listed first, so all BIG (1e30) entries live in a 128-wide window at
    each segment start; a small extra matmul per window accumulates the
    per-tile diagmask (BIG at each query's own column) through
    lhsT = identity, start=False.  Position is data, not code.
  - min: DVE tensor_reduce(min) per slot; counts: per type segment a
    ScalarE Sign pass (accum_out; count = (S + W)/2; self/pads give -1 and
    drop out) or a DVE is_lt pass (count = S) - the segment->engine map
    balances ACT and DVE.
  - Host finalizes: count formula, inverse permutations, max; any query
    whose candidate min is >= RCUT^2 (nearest neighbour outside the
    candidate radius; does not occur at realistic densities) is recomputed
    exactly on host.
"""
import sys

sys.path.insert(0, "/opt/trn_rl_repo")

import numpy as np
import ml_dtypes

NFRAMES = 2
NLOC = 4096
NTYPES = 4
RCUT = 6.0
RCUT2 = 36.0
CENTER = 20.0
BIG = 1.0e30
PAD_COORD = 1000.0
NCORES = 8
CPF = 4                       # cores per frame
QPC = NLOC // CPF             # queries per core = 1024
NQT = QPC // 128              # query tiles (slots) per core = 8
NTILES = NLOC // 128          # query tiles per frame = 32
PAIRS = [(0, 0), (0, 1), (1, 0), (1, 1), (0, 2), (2, 0)]
K = 36                        # 12 split rows per component


def seg_engine(t, tau):
    """Engine for the count pass of slot t, type-segment tau (balance)."""
    return "D" if tau == 3 or (tau == 2 and t >= 4) else "A"

_CACHE = {}


def _split3(x):
    p1 = x.astype(ml_dtypes.bfloat16)
    r1 = (x.astype(np.float64) - p1.astype(np.float64)).astype(np.float32)
    p2 = r1.astype(ml_dtypes.bfloat16)
    r2 = (r1.astype(np.float64) - p2.astype(np.float64)).astype(np.float32)
    p3 = r2.astype(ml_dtypes.bfloat16)
    return [p1, p2, p3]


def _features(coords):
    """coords: (n, 3) float64 centered.
    Returns (qfeat (K, n), kfeat (K, n)) as bfloat16 arrays."""
    n = len(coords)
    ones = np.ones(n, ml_dtypes.bfloat16)
    L, R = [], []
    for d in range(3):
        qc = coords[:, d]
        q2 = (qc * qc).astype(np.float32)
        q2p = _split3(q2)
        qp = _split3(qc.astype(np.float32))
        m2p = _split3((-2.0 * qc).astype(np.float32))
        for i in range(3):              # qc^2 pieces x 1
            L.append(q2p[i]); R.append(ones)
        for (ia, ib) in PAIRS:          # qc pieces x -2kc pieces
            L.append(qp[ia]); R.append(m2p[ib])
        for i in range(3):              # 1 x kc^2 pieces
            L.append(ones); R.append(q2p[i])
    return (np.stack(L).astype(ml_dtypes.bfloat16),
            np.stack(R).astype(ml_dtypes.bfloat16))


def _kd_perm(c):
    """c: (n, 3) raw coords. Recursive median split into 128-atom leaves."""
    def split(ids):
        if len(ids) <= 128:
            return [ids]
        spread = c[ids].max(0) - c[ids].min(0)
        ax = int(np.argmax(spread))
        order = ids[np.argsort(c[ids, ax], kind="stable")]
        h = len(ids) // 2
        return split(order[:h]) + split(order[h:])
    return np.concatenate(split(np.arange(len(c))))


def _build(wslots, repeat=1):
    """Build + lower the SPMD kernel. wslots: per-slot segment widths."""
    import concourse.bacc as bacc
    import concourse.tile as tile
    from concourse import mybir

    f32 = mybir.dt.float32
    bf16 = mybir.dt.bfloat16
    Ws = [4 * w for w in wslots]
    col0 = np.concatenate([[0], np.cumsum(Ws)])      # kfeat col offset per slot
    WTOT = int(col0[-1])
    WBMAX = ((max(Ws) + 511) // 512) * 512

    nc = bacc.Bacc("TRN2", target_bir_lowering=False, debug=False,
                   num_devices=NCORES)
    qf = nc.dram_tensor("qfeat", [K, QPC], bf16, kind="ExternalInput").ap()
    kf = nc.dram_tensor("kfeat", [K, WTOT], bf16, kind="ExternalInput").ap()
    dm = nc.dram_tensor("diagmask", [128, NQT * NTYPES * 128], bf16,
                        kind="ExternalInput").ap()
    idn = nc.dram_tensor("ident", [128, 128], bf16, kind="ExternalInput").ap()
    out_min = nc.dram_tensor("out_min", [128, NQT], f32,
                             kind="ExternalOutput").ap()
    out_cnt = nc.dram_tensor("out_cnt", [128, NQT * NTYPES], f32,
                             kind="ExternalOutput").ap()

    wide = [t for t in range(NQT) if 4 * wslots[t] > 512]
    narrow = [t for t in range(NQT) if 4 * wslots[t] <= 512]
    WBWIDE = ((max((4 * wslots[t] for t in wide), default=512) + 511)
              // 512) * 512

    with tile.TileContext(nc) as tc:
        with (
            tc.tile_pool(name="singles", bufs=1) as singles,
            tc.tile_pool(name="sc_act", bufs=3) as sc_act_pool,
            tc.tile_pool(name="sc_dve", bufs=2) as sc_dve_pool,
        ):
            qsb = singles.tile([K, QPC], bf16)
            nc.sync.dma_start(out=qsb[:], in_=qf)
            ksb = singles.tile([K, WTOT], bf16)
            dmsb = singles.tile([128, NQT * NTYPES * 128], bf16)
            for t in range(NQT):       # per-slot DMAs so slot 0 starts early
                a, b = int(col0[t]), int(col0[t + 1])
                nc.sync.dma_start(out=ksb[:, a:b], in_=kf[:, a:b])
                da, db = t * NTYPES * 128, (t + 1) * NTYPES * 128
                nc.sync.dma_start(out=dmsb[:, da:db], in_=dm[:, da:db])
            idsb = singles.tile([128, 128], bf16)
            nc.sync.dma_start(out=idsb[:], in_=idn)
            bias36 = singles.tile([128, 1], f32)
            nc.vector.memset(bias36[:], RCUT2)
            min_sb = singles.tile([128, NQT], f32)
            cnt_sb = singles.tile([128, NQT * NTYPES], f32)

            def do_slot(t, psum_pool, wb):
                w = wslots[t]
                W = 4 * w
                k0 = int(col0[t])
                ps = psum_pool.tile([128, wb], f32, tag="ps")
                # diag windows: [tau*w, tau*w + min(128, w)) split at banks;
                # (ps range, dmask col) pieces, coalesced when contiguous in
                # both ps and dmsb (always the case when w == 128)
                pieces = []
                for tau in range(NTYPES):
                    a = tau * w
                    b = a + min(128, w)
                    dc = (t * NTYPES + tau) * 128
                    while a < b:
                        e = min(b, ((a // 512) + 1) * 512)
                        pieces.append([a, e, dc])
                        dc += e - a
                        a = e
                merged = [pieces[0]]
                for a, e, dc in pieces[1:]:
                    pa, pe, pdc = merged[-1]
                    if (a == pe and dc == pdc + (pe - pa)
                            and (a % 512) != 0):
                        merged[-1][1] = e
                    else:
                        merged.append([a, e, dc])
                by_bank = {}
                for a, e, dc in merged:
                    by_bank.setdefault(a // 512, []).append((a, e, dc))
                for b0 in range(0, W, 512):
                    b1 = min(b0 + 512, W)
                    nc.tensor.matmul(
                        ps[:, b0:b1],
                        lhsT=qsb[:, t * 128:(t + 1) * 128],
                        rhs=ksb[:, k0 + b0:k0 + b1],
                        start=True, stop=(b0 // 512) not in by_bank,
                    )
                for bank in sorted(by_bank):
                    subs = by_bank[bank]
                    for i, (a, e, dcol) in enumerate(subs):
                        nc.tensor.matmul(
                            ps[:, a:e],
                            lhsT=idsb[:],
                            rhs=dmsb[:, dcol:dcol + (e - a)],
                            start=False, stop=(i == len(subs) - 1),
                            skip_group_check=True,
                        )
                nc.vector.tensor_reduce(
                    out=min_sb[:, t:t + 1], in_=ps[:, 0:W],
                    axis=mybir.AxisListType.X, op=mybir.AluOpType.min,
                )
                for s in range(NTYPES):
                    a, b_ = s * w, (s + 1) * w
                    ccol = t * NTYPES + s
                    if seg_engine(t, s) == "A":
                        sc = sc_act_pool.tile([128, WBWIDE], f32, tag="sa")
                        nc.scalar.activation(
                            sc[:, a:b_], ps[:, a:b_],
                            mybir.ActivationFunctionType.Sign,
                            bias=bias36[:], scale=-1.0,
                            accum_out=cnt_sb[:, ccol:ccol + 1],
                        )
                    else:
                        sc = sc_dve_pool.tile([128, WBWIDE], f32, tag="sd")
                        nc.vector.tensor_scalar(
                            out=sc[:, a:b_], in0=ps[:, a:b_],
                            scalar1=RCUT2, scalar2=None,
                            op0=mybir.AluOpType.is_lt,
                            op1=mybir.AluOpType.add,
                            accum_out=cnt_sb[:, ccol:ccol + 1],
                        )

            def body(_iv=None):
                with tc.tile_pool(name="psA", bufs=2, space="PSUM") as pA:
                    for t in wide:
                        do_slot(t, pA, WBWIDE)
                with tc.tile_pool(name="psB", bufs=max(2, len(narrow)),
                                  space="PSUM") as pB:
                    for t in narrow:
                        do_slot(t, pB, 512)

            if repeat == 1:
                body()
            else:
                with tc.For_i(0, repeat, 1) as iv:
                    body(iv)

            nc.sync.dma_start(out=out_min, in_=min_sb[:])
            nc.sync.dma_start(out=out_cnt, in_=cnt_sb[:])

    nc.compile()
    return nc


def _prep(coord, atype):
    """Host-side prep.

    Returns (in_maps, perms, tile_orders, wslots)."""
    c = np.asarray(coord, dtype=np.float32).reshape(NFRAMES, NLOC, 3)
    at = np.asarray(atype)

    perms, atsorted, csorted = [], [], []
    cand = [[None] * NTILES for _ in range(NFRAMES)]
    for f in range(NFRAMES):
        perm = _kd_perm(c[f])
        perms.append(perm)
        cs = c[f][perm]
        ats = at[f][perm]
        csorted.append(cs)
        atsorted.append(ats)
        cs64 = cs.astype(np.float64)
        for tg in range(NTILES):
            q = cs64[tg * 128:(tg + 1) * 128]
            lo, hi = q.min(0), q.max(0)
            d = np.maximum(np.maximum(lo - cs64, cs64 - hi), 0.0)
            idx = np.nonzero((d * d).sum(1) <= (RCUT + 0.01) ** 2)[0]
            # tile's own atoms first within each type (diagmask window)
            own = (idx >= tg * 128) & (idx < (tg + 1) * 128)
            bytype = []
            for tt in range(NTYPES):
                sel = idx[ats[idx] == tt]
                o = sel[(sel >= tg * 128) & (sel < (tg + 1) * 128)]
                rest = sel[(sel < tg * 128) | (sel >= (tg + 1) * 128)]
                bytype.append(np.concatenate([o, rest]))
            cand[f][tg] = bytype

    # processing order: per core, tiles sorted by type-max width descending
    tile_orders = []       # per core: list of global tile ids in slot order
    for core in range(NCORES):
        f, s = core // CPF, core % CPF
        tiles = list(range(s * NQT, (s + 1) * NQT))
        tiles.sort(key=lambda tg: -max(len(b) for b in cand[f][tg]))
        tile_orders.append(tiles)
    wslots = []
    for t in range(NQT):
        wmax = 0
        for core in range(NCORES):
            f = core // CPF
            tg = tile_orders[core][t]
            wmax = max(wmax, max(len(b) for b in cand[f][tg]))
        wslots.append(max(128, ((wmax + 31) // 32) * 32))
    Ws = [4 * w for w in wslots]
    col0 = np.concatenate([[0], np.cumsum(Ws)]).astype(int)
    WTOT = int(col0[-1])

    ident = np.eye(128, dtype=ml_dtypes.bfloat16)

    in_maps = []
    for core in range(NCORES):
        f, s = core // CPF, core % CPF
        cs64 = csorted[f].astype(np.float64) - CENTER
        qfeat_all, kfeat_all = _features(cs64)
        padf = _features(np.full((1, 3), PAD_COORD - CENTER))[1][:, 0]
        kfeat = np.empty((K, WTOT), ml_dtypes.bfloat16)
        dmask = np.zeros((128, NQT * NTYPES * 128), ml_dtypes.bfloat16)
        qfeat = np.empty((K, QPC), ml_dtypes.bfloat16)
        for t in range(NQT):
            tg = tile_orders[core][t]
            w = wslots[t]
            qfeat[:, t * 128:(t + 1) * 128] = \
                qfeat_all[:, tg * 128:(tg + 1) * 128]
            for tt in range(NTYPES):
                ids = cand[f][tg][tt]
                n = len(ids)
                seg = int(col0[t]) + tt * w
                kfeat[:, seg:seg + n] = kfeat_all[:, ids]
                kfeat[:, seg + n:seg + w] = padf[:, None]
                # own atoms sit at positions 0..m-1 of this segment
                for j, a_ in enumerate(ids):
                    if tg * 128 <= a_ < (tg + 1) * 128:
                        p = int(a_) - tg * 128
                        dmask[p, (t * NTYPES + tt) * 128 + j] = BIG
        in_maps.append({
            "qfeat": qfeat,
            "kfeat": kfeat,
            "diagmask": dmask,
            "ident": ident,
        })
    return in_maps, perms, tile_orders, wslots


def _postprocess(results, perms, tile_orders, wslots, coord, atype):
    c = np.asarray(coord, dtype=np.float32).reshape(NFRAMES, NLOC, 3)
    min_rr2 = np.empty((NFRAMES, NLOC), np.float32)
    max_nnei = np.empty((NFRAMES, NTYPES), np.int64)
    nnei_max = np.zeros((NFRAMES, NTYPES), np.int64)
    mins_sorted = [np.empty(NLOC, np.float32) for _ in range(NFRAMES)]
    for core in range(NCORES):
        f, s = core // CPF, core % CPF
        r = results[core]
        mn = r["out_min"]                      # (128, NQT)
        cnt = r["out_cnt"].reshape(128, NQT, NTYPES)
        for t in range(NQT):
            tg = tile_orders[core][t]
            mins_sorted[f][tg * 128:(tg + 1) * 128] = mn[:, t]
            for tt in range(NTYPES):
                if seg_engine(t, tt) == "A":
                    lt = (cnt[:, t, tt] + wslots[t]) * 0.5
                else:
                    lt = cnt[:, t, tt]
                nnei_max[f, tt] = max(nnei_max[f, tt],
                                      int(np.round(lt.max())))
    for f in range(NFRAMES):
        ms = mins_sorted[f]
        bad = np.nonzero(ms >= RCUT2)[0]
        if len(bad):
            cs = c[f][perms[f]].astype(np.float32)
            for i in bad:
                d = cs - cs[i]
                rr = (d[:, 0] * d[:, 0] + d[:, 1] * d[:, 1]
                      + d[:, 2] * d[:, 2]).astype(np.float32)
                rr[i] = np.inf
                ms[i] = rr.min()
        min_rr2[f, perms[f]] = ms
        max_nnei[f] = nnei_max[f]
    return min_rr2, max_nnei.astype(np.int32)


def kernel(coord, atype):
    from concourse.bass_utils import run_bass_kernel_spmd

    in_maps, perms, tile_orders, wslots = _prep(coord, atype)
    key = tuple(wslots)
    if key not in _CACHE:
        _CACHE[key] = _build(wslots)
    nc = _CACHE[key]
    res = run_bass_kernel_spmd(nc, in_maps, list(range(NCORES)))
    return _postprocess(res.results, perms, tile_orders, wslots,
                        coord, atype)
